# revision 2
# baseline (speedup 1.0000x reference)
"""Block-circulant linear (MINI_BLOCK=4) via length-4 rFFT factorization, v8.

Math: out = x @ W^T where W[4y+n, 4x+j] = eigens[y, x, (n-j) mod 4].
Length-4 DFT domain, Gauss 3-mult for the complex bin -> 5 real matmul
chains over gx=1024:
  X0 = x0+x1+x2+x3, X1 = (x0-x2) + i(x3-x1), X2 = x0-x1+x2-x3
  Y0 = X0 E0, Y2 = X2 E2, g1 = (X1r+X1i)E1r, g2 = X1r(E1i-E1r), g3 = X1i(E1r+E1i)
  Y1r = g1-g3, Y1i = g1+g2
  o0 = Y0+Y1r+Y2, o1 = Y0-Y1i-Y2, o2 = Y0-Y1r+Y2, o3 = Y0+Y1i-Y2

v5 notes (evidence-driven, from NTFF traces):
- Per-core aggregate DMA is ~250GB/s no matter how many queues, so total
  bytes rule the prologue. Ship only 4 E tensors (8MB) and derive
  Ed=E1i-E1r / Es=E1r+E1i on DVE per z-slice (+0.9us each, DVE has slack).
- TRN2 PE p-state: 0.65 -> 1.2 -> 2.4GHz, max only after ~3us of
  CONTINUOUS execution; any idle gap resets it. A warmup chain of dummy
  matmuls starts at the init barrier, and filler matmuls pad the
  DMA-starved xc-steps of z0/z1 so the clock never drops.
- Feature-major matmuls (stationary E [128x,128y], moving X [128x,512b]),
  contiguous bf16 drains (DVE strided writes are 4.5x slower), output
  transposed [feature, batch] fixed on host.
- k-major chain order for z>=2 with drains interleaved mid-group: only
  ~3us of combines left after the final matmul.
- Outs ride SWDGE (Pool) as 4x128KB per z; z6/z7 ride the HWDGE queues
  (idle by then).
"""
import numpy as np

B, IN, OUT, BLK = 4096, 4096, 4096, 4
GX, GY = IN // BLK, OUT // BLK        # 1024, 1024
NCORES = 8
BS = B // NCORES                      # 512 batch rows per core
XC = GX // 128                        # 8 contraction chunks
YZ = GY // 128                        # 8 output z-groups (128 y each)

_cache = {}

# bins: 0=Y0(E0*X0) 1=g1(E1r*X1s) 2=g2(Ed*X1r) 3=Y2(E2*X2) 4=g3(Es*X1i)
PS_BUFS = {0: 1, 1: 1, 2: 2, 3: 2, 4: 1}   # + 1 filler bank = 8 total
KORDER = (3, 2, 4, 0, 1)              # z0/z1 xc-major step order
KORDER_K = (0, 3, 1, 2, 4)            # k-major groups; drains interleave
WARMUP = 16                           # dummy matmuls before first real chain
FILL0, FILL1, FILLB = 2, 1, 1         # fillers per xc-step (z0, z1, boundary)


def _build_nc():
    from concourse import bacc
    import concourse.mybir as mybir
    from concourse.tile import TileContext

    f32 = mybir.dt.float32
    bf16 = mybir.dt.bfloat16

    nc = bacc.Bacc("TRN2", target_bir_lowering=False, debug=False,
                   enable_asserts=False, num_devices=NCORES)
    x_d = [nc.dram_tensor(nm, [XC // 2, 128, 2, BS], bf16, kind="ExternalInput")
           for nm in ("x0", "x1r", "x1i", "x2")]
    e_d = [nc.dram_tensor(nm, [YZ, 128, XC, 128], bf16, kind="ExternalInput")
           for nm in ("e0", "e1r", "e1i", "e2")]
    out_d = nc.dram_tensor("outT", [OUT, BS], bf16, kind="ExternalOutput")

    with TileContext(nc) as tc:
        with (
            tc.tile_pool(name="xp", bufs=1) as xp,
            tc.tile_pool(name="ep", bufs=1) as ep,
            tc.tile_pool(name="dr", bufs=2) as dr,
            tc.tile_pool(name="op", bufs=4) as op_,
            tc.tile_pool(name="ps", bufs=1, space="PSUM") as ps,
        ):
            # ---- warmup: spin the PE p-state up while DMA streams in -----
            wt = xp.tile([128, BS], bf16, tag="wt")
            wpsum = ps.tile([128, BS], f32, tag="psf")
            nc.gpsimd.memset(wt, 0)
            for r in range(WARMUP):
                nc.tensor.matmul(wpsum, wt[:, 0:128], wt,
                                 start=r == 0, stop=r == WARMUP - 1)

            def filler(n):
                for r in range(n):
                    nc.tensor.matmul(wpsum, wt[:, 0:128], wt,
                                     start=r == 0, stop=r == n - 1)

            # ---- async loads (both HWDGE queues, need-order) --------------
            xpair = [[xp.tile([128, 2, BS], bf16, tag=f"xp{k}_{g}",
                               name=f"xp{k}_{g}") for g in range(XC // 2)]
                     for k in range(5)]  # k=1 slot is X1s (derived)
            xt = [[xpair[k][c // 2][:, c % 2] for c in range(XC)]
                  for k in range(5)]
            # et: k0=E0,k1=E1r,k2=Ed(derived),k3=E2,k4=Es(derived); el2=E1i
            et = [[ep.tile([128, XC, 128], bf16, tag=f"e{k}_{z}",
                           name=f"e{k}_{z}") for z in range(YZ)]
                  for k in range(5)]
            el2 = [ep.tile([128, XC, 128], bf16, tag=f"ei_{z}", name=f"ei_{z}")
                   for z in range(YZ)]

            def eload(k, z):       # k indexes (e0, e1r, e1i, e2)
                dst = el2[z] if k == 2 else et[k][z]
                return (dst, e_d[k][z])

            loads = [eload(3, 0), eload(1, 0), eload(2, 0), eload(0, 0)]
            loads += [(xpair[3][0], x_d[3][0]), (xpair[2][0], x_d[1][0]),
                      (xpair[4][0], x_d[2][0]), (xpair[0][0], x_d[0][0])]
            ez = [[eload(kk, z) for kk in ((3, 1, 2, 0) if z < 2
                                           else (0, 3, 1, 2))]
                  for z in range(YZ)]
            for g in range(1, XC // 2):                # X pairs + E z1/z2 JIT
                loads += [(xpair[3][g], x_d[3][g]), (xpair[2][g], x_d[1][g]),
                          (xpair[4][g], x_d[2][g]), (xpair[0][g], x_d[0][g])]
                if g == 1:
                    loads += ez[1]
                elif g == 2:
                    loads += ez[2]
            # scalar (ACT) issues NO loads: HWDGE dma_start backpressure
            # on the ACT engine stream would delay the PSUM copies that
            # gate the single-buffered banks (seen as a 9us PE gap in v5).
            # Head (Ez0-2 + X) ping-pongs sync/SWDGE; late E slices ride
            # sync alone (it drains its head share by ~22us and lands z7
            # by ~55us, 20us before needed; SWDGE then turns to outs).
            hw = (nc.sync, nc.gpsimd)
            for i, (dst, src) in enumerate(loads):
                hw[i % 2].dma_start(out=dst, in_=src)
            for z in range(3, YZ):
                for dst, src in ez[z]:
                    nc.sync.dma_start(out=dst, in_=src)

            # early DVE work: Ed/Es for z0/z1, X1s per chunk
            def derive(z):
                nc.vector.tensor_sub(out=et[2][z], in0=el2[z], in1=et[1][z])
                nc.vector.tensor_add(out=et[4][z], in0=et[1][z], in1=el2[z])

            derive(0)
            for g in range(XC // 2):
                nc.vector.tensor_add(out=xpair[1][g], in0=xpair[2][g],
                                     in1=xpair[4][g])
            derive(1)

            # ---- main: 5 chains per z-group, inverse DFT, store ------------
            for z in range(YZ):
                pst = {k: ps.tile([128, BS], f32, tag=f"ps{k}", name=f"ps{k}",
                                  bufs=PS_BUFS[k]) for k in range(5)}
                t_ = dr.tile([128, BS], bf16, tag="t")
                v_ = dr.tile([128, BS], bf16, tag="v")
                a_ = dr.tile([128, BS], bf16, tag="a")
                b_ = dr.tile([128, BS], bf16, tag="b")
                c_ = dr.tile([128, BS], bf16, tag="c")
                d_ = dr.tile([128, BS], bf16, tag="d")
                ot = op_.tile([128, 4, BS], bf16, tag="ot")
                ov = out_d[:, :].rearrange("(z p j) b -> z p j b", p=128, j=4)[z]
                oeng = {6: (nc.sync, nc.scalar, nc.sync, nc.scalar),
                        7: (nc.scalar, nc.sync, nc.scalar, nc.sync)}.get(
                    z, (nc.gpsimd,) * 4)

                if z < 2:
                    for c in range(XC):
                        st, sp = c == 0, c == XC - 1
                        for k in KORDER:
                            nc.tensor.matmul(pst[k], et[k][z][:, c], xt[k][c],
                                             start=st, stop=sp)
                        filler(FILL0 if z == 0 else FILL1)
                    filler(FILLB)
                else:
                    # KORDER_K = (0,3,1,2,4); each chain's dependent drain
                    # ops fire right after it, so after the FINAL chain only
                    # c, o0, o2 (~1.4us DVE) remain.
                    for k in KORDER_K:
                        for c in range(XC):
                            nc.tensor.matmul(pst[k], et[k][z][:, c], xt[k][c],
                                             start=c == 0, stop=c == XC - 1)
                        if k == 0:
                            nc.scalar.copy(out=t_, in_=pst[0])    # Y0
                        elif k == 3:
                            nc.vector.tensor_add(out=a_, in0=t_, in1=pst[3])
                            nc.vector.tensor_sub(out=b_, in0=t_, in1=pst[3])
                        elif k == 1:
                            nc.scalar.copy(out=v_, in_=pst[1])    # g1
                        elif k == 2:
                            nc.vector.tensor_add(out=d_, in0=v_, in1=pst[2])
                            nc.vector.tensor_sub(out=ot[:, 1], in0=b_, in1=d_)
                            nc.vector.tensor_add(out=ot[:, 3], in0=b_, in1=d_)
                            if z >= 6:
                                oeng[1].dma_start(out=ov[:, 1], in_=ot[:, 1])
                                oeng[3].dma_start(out=ov[:, 3], in_=ot[:, 3])
                    nc.vector.tensor_sub(out=c_, in0=v_, in1=pst[4])
                    nc.vector.tensor_add(out=ot[:, 0], in0=a_, in1=c_)
                    nc.vector.tensor_sub(out=ot[:, 2], in0=a_, in1=c_)
                    if z < 6:
                        nc.gpsimd.dma_start(out=ov, in_=ot)
                    else:
                        oeng[0].dma_start(out=ov[:, 0], in_=ot[:, 0])
                        oeng[2].dma_start(out=ov[:, 2], in_=ot[:, 2])
                if z < 2:
                    nc.scalar.copy(out=t_, in_=pst[0])
                    nc.scalar.copy(out=v_, in_=pst[1])
                    nc.vector.tensor_sub(out=c_, in0=v_, in1=pst[4])
                    nc.vector.tensor_add(out=d_, in0=v_, in1=pst[2])
                    nc.vector.tensor_add(out=a_, in0=t_, in1=pst[3])
                    nc.vector.tensor_add(out=ot[:, 0], in0=a_, in1=c_)
                    nc.vector.tensor_sub(out=b_, in0=t_, in1=pst[3])
                    nc.vector.tensor_sub(out=ot[:, 1], in0=b_, in1=d_)
                    nc.vector.tensor_sub(out=ot[:, 2], in0=a_, in1=c_)
                    nc.vector.tensor_add(out=ot[:, 3], in0=b_, in1=d_)
                    nc.gpsimd.dma_start(out=ov, in_=ot)
                if z + 2 < YZ:
                    derive(z + 2)
    nc.compile()
    return nc


def _prep_eigens(eigens):
    """eigens (gy, gx, 4) -> four (YZ, 128, XC, 128) bf16 E-matrices
    (E0, E1r, E1i, E2), [x, y] oriented, irfft scales folded."""
    e = np.ascontiguousarray(eigens.transpose(1, 0, 2)).astype(np.float32)  # (x,y,j)
    e0 = ((e[..., 0] + e[..., 2]) + (e[..., 1] + e[..., 3])) * 0.25
    e2 = ((e[..., 0] + e[..., 2]) - (e[..., 1] + e[..., 3])) * 0.25
    e1r = (e[..., 0] - e[..., 2]) * 0.5
    e1i = (e[..., 3] - e[..., 1]) * 0.5

    import ml_dtypes

    def chunk(m):  # (GX, GY) -> (YZ, 128, XC, 128): [z, p=x%128, c=x//128, q]
        return np.ascontiguousarray(
            m.reshape(XC, 128, YZ, 128).transpose(2, 1, 0, 3)
        ).astype(ml_dtypes.bfloat16)
    return (chunk(e0), chunk(e1r), chunk(e1i), chunk(e2))


def _prep_x(x):
    """x (B, IN) f32 -> four (XC, 128, B) bf16 bins (full batch; sliced
    per core in _in_maps)."""
    import ml_dtypes
    xb = np.ascontiguousarray(x, dtype=np.float32).reshape(B, GX, BLK)
    s02 = xb[:, :, 0] + xb[:, :, 2]
    s13 = xb[:, :, 1] + xb[:, :, 3]
    bins = (s02 + s13, xb[:, :, 0] - xb[:, :, 2],
            xb[:, :, 3] - xb[:, :, 1], s02 - s13)   # X0, X1r, X1i, X2
    return [np.ascontiguousarray(
                b.T.reshape(XC // 2, 2, 128, B).transpose(0, 2, 1, 3)
            ).astype(ml_dtypes.bfloat16) for b in bins]


def _in_maps(x, eigens):
    e0, e1r, e1i, e2 = _prep_eigens(np.asarray(eigens))
    x0, x1r, x1i, x2 = _prep_x(np.asarray(x))
    return [
        {"x0": np.ascontiguousarray(x0[:, :, :, c * BS:(c + 1) * BS]),
         "x1r": np.ascontiguousarray(x1r[:, :, :, c * BS:(c + 1) * BS]),
         "x1i": np.ascontiguousarray(x1i[:, :, :, c * BS:(c + 1) * BS]),
         "x2": np.ascontiguousarray(x2[:, :, :, c * BS:(c + 1) * BS]),
         "e0": e0, "e1r": e1r, "e1i": e1i, "e2": e2}
        for c in range(NCORES)
    ]


def _assemble(results):
    return np.concatenate(
        [np.asarray(r["outT"]).astype(np.float32).T for r in results], axis=0)


def kernel(x, eigens):
    from concourse.bass_utils import run_bass_kernel_spmd

    if "nc" not in _cache:
        _cache["nc"] = _build_nc()
    res = run_bass_kernel_spmd(_cache["nc"], _in_maps(x, eigens),
                               core_ids=list(range(NCORES)))
    return _assemble(res.results)


# revision 3
# speedup vs baseline: 1.0623x; 1.0623x over previous
"""Block-circulant linear (MINI_BLOCK=4) via length-4 rFFT factorization, v9.

Math: out = x @ W^T where W[4y+n, 4x+j] = eigens[y, x, (n-j) mod 4].
Length-4 DFT domain, Gauss 3-mult for the complex bin -> 5 real matmul
chains over gx=1024:
  X0 = x0+x1+x2+x3, X1 = (x0-x2) + i(x3-x1), X2 = x0-x1+x2-x3
  Y0 = X0 E0, Y2 = X2 E2, g1 = (X1r+X1i)E1r, g2 = X1r(E1i-E1r), g3 = X1i(E1r+E1i)
  Y1r = g1-g3, Y1i = g1+g2
  o0 = Y0+Y1r+Y2, o1 = Y0-Y1i-Y2, o2 = Y0-Y1r+Y2, o3 = Y0+Y1i-Y2

v5 notes (evidence-driven, from NTFF traces):
- Per-core aggregate DMA is ~250GB/s no matter how many queues, so total
  bytes rule the prologue. Ship only 4 E tensors (8MB) and derive
  Ed=E1i-E1r / Es=E1r+E1i on DVE per z-slice (+0.9us each, DVE has slack).
- TRN2 PE p-state: 0.65 -> 1.2 -> 2.4GHz, max only after ~3us of
  CONTINUOUS execution; any idle gap resets it. A warmup chain of dummy
  matmuls starts at the init barrier, and filler matmuls pad the
  DMA-starved xc-steps of z0/z1 so the clock never drops.
- Feature-major matmuls (stationary E [128x,128y], moving X [128x,512b]),
  contiguous bf16 drains (DVE strided writes are 4.5x slower), output
  transposed [feature, batch] fixed on host.
- k-major chain order for z>=2 with drains interleaved mid-group: only
  ~3us of combines left after the final matmul.
- Outs ride SWDGE (Pool) as 4x128KB per z; z6/z7 ride the HWDGE queues
  (idle by then).
"""
import numpy as np

B, IN, OUT, BLK = 4096, 4096, 4096, 4
GX, GY = IN // BLK, OUT // BLK        # 1024, 1024
NCORES = 8
BS = B // NCORES                      # 512 batch rows per core
XC = GX // 128                        # 8 contraction chunks
YZ = GY // 128                        # 8 output z-groups (128 y each)

_cache = {}

# bins: 0=Y0(E0*X0) 1=g1(E1r*X1s) 2=g2(Ed*X1r) 3=Y2(E2*X2) 4=g3(Es*X1i)
PS_BUFS = {0: 1, 1: 2, 2: 1, 3: 2, 4: 1}   # + 1 filler bank = 8 total
KORDER = (3, 2, 4, 0, 1)              # z0/z1 xc-major step order
KORDER_K = (0, 3, 1, 2, 4)            # k-major groups; drains interleave
WARMUP = 16                           # dummy matmuls before first real chain
FILL0, FILL1, FILLB = 1, 1, 1         # fillers per xc-step (z0, z1, boundary)


def _build_nc():
    from concourse import bacc
    import concourse.mybir as mybir
    from concourse.tile import TileContext

    f32 = mybir.dt.float32
    bf16 = mybir.dt.bfloat16

    nc = bacc.Bacc("TRN2", target_bir_lowering=False, debug=False,
                   enable_asserts=False, num_devices=NCORES)
    x_d = [nc.dram_tensor(nm, [XC // 2, 128, 2, BS], bf16, kind="ExternalInput")
           for nm in ("x0", "x1r", "x1i", "x2")]
    e_d = [nc.dram_tensor(nm, [YZ, 128, XC, 128], bf16, kind="ExternalInput")
           for nm in ("e0", "e1r", "e1i", "e2")]
    out_d = nc.dram_tensor("outT", [OUT, BS], bf16, kind="ExternalOutput")

    with TileContext(nc) as tc:
        with (
            tc.tile_pool(name="xp", bufs=1) as xp,
            tc.tile_pool(name="ep", bufs=1) as ep,
            tc.tile_pool(name="dr", bufs=2) as dr,
            tc.tile_pool(name="op", bufs=4) as op_,
            tc.tile_pool(name="ps", bufs=1, space="PSUM") as ps,
        ):
            # ---- warmup: spin the PE p-state up while DMA streams in -----
            wt = xp.tile([128, BS], bf16, tag="wt")
            wpsum = ps.tile([128, BS], f32, tag="psf")
            nc.gpsimd.memset(wt, 0)
            for r in range(WARMUP):
                nc.tensor.matmul(wpsum, wt[:, 0:128], wt,
                                 start=r == 0, stop=r == WARMUP - 1)

            def filler(n):
                for r in range(n):
                    nc.tensor.matmul(wpsum, wt[:, 0:128], wt,
                                     start=r == 0, stop=r == n - 1)

            # ---- async loads (both HWDGE queues, need-order) --------------
            xpair = [[xp.tile([128, 2, BS], bf16, tag=f"xp{k}_{g}",
                               name=f"xp{k}_{g}") for g in range(XC // 2)]
                     for k in range(5)]  # k=1 slot is X1s (derived)
            xt = [[xpair[k][c // 2][:, c % 2] for c in range(XC)]
                  for k in range(5)]
            # et: k0=E0,k1=E1r,k2=Ed(derived),k3=E2,k4=Es(derived); el2=E1i
            et = [[ep.tile([128, XC, 128], bf16, tag=f"e{k}_{z}",
                           name=f"e{k}_{z}") for z in range(YZ)]
                  for k in range(5)]
            el2 = [ep.tile([128, XC, 128], bf16, tag=f"ei_{z}", name=f"ei_{z}")
                   for z in range(YZ)]

            def eload(k, z):       # k indexes (e0, e1r, e1i, e2)
                dst = el2[z] if k == 2 else et[k][z]
                return (dst, e_d[k][z])

            ez = [[eload(kk, z) for kk in ((3, 1, 2, 0) if z < 2
                                           else (0, 3, 1, 2))]
                  for z in range(YZ)]
            loads = [eload(3, 0), eload(1, 0), eload(2, 0), eload(0, 0)]
            loads += [(xpair[3][0], x_d[3][0]), (xpair[2][0], x_d[1][0]),
                      (xpair[4][0], x_d[2][0]), (xpair[0][0], x_d[0][0])]
            loads += ez[1][:2]                         # E2/E1r of z1 (interleaved bins)
            for g in range(1, XC // 2):                # X pairs + E z1/z2 JIT
                loads += [(xpair[3][g], x_d[3][g]), (xpair[2][g], x_d[1][g]),
                          (xpair[4][g], x_d[2][g]), (xpair[0][g], x_d[0][g])]
                if g == 1:
                    loads += ez[1][2:]
                elif g == 2:
                    loads += ez[2]
            # scalar (ACT) issues NO loads: HWDGE dma_start backpressure
            # on the ACT engine stream would delay the PSUM copies that
            # gate the single-buffered banks (seen as a 9us PE gap in v5).
            # Head (Ez0-2 + X) ping-pongs sync/SWDGE; late E slices ride
            # sync alone (it drains its head share by ~22us and lands z7
            # by ~55us, 20us before needed; SWDGE then turns to outs).
            hw = (nc.sync, nc.gpsimd)
            for i, (dst, src) in enumerate(loads):
                hw[i % 2].dma_start(out=dst, in_=src)
            for z in range(3, YZ):
                for dst, src in ez[z]:
                    nc.sync.dma_start(out=dst, in_=src)

            # early DVE work: Ed/Es for z0/z1, X1s per chunk
            def derive(z):
                nc.vector.tensor_sub(out=et[2][z], in0=el2[z], in1=et[1][z])
                nc.vector.tensor_add(out=et[4][z], in0=et[1][z], in1=el2[z])

            derive(0)
            for g in range(XC // 2):
                nc.vector.tensor_add(out=xpair[1][g], in0=xpair[2][g],
                                     in1=xpair[4][g])
            derive(1)

            # ---- main ------------------------------------------------------
            # Prologue: z0's 5 chains PLUS z1's two double-buffered bins
            # (y2, g1 -- loaded E only, no derives) interleave in one xc
            # sweep: 7 matmuls per chunk matches the ~3us/pair X arrival,
            # so the PE stays fed while X streams in. z1 finishes its
            # remaining 3 chains right after. z>=2: k-major, drains
            # interleaved mid-group.
            pstore = {}

            def pst(z, k):
                if (z, k) not in pstore:
                    pstore[(z, k)] = ps.tile([128, BS], f32, tag=f"ps{k}",
                                             name=f"ps{z}_{k}",
                                             bufs=PS_BUFS[k])
                return pstore[(z, k)]

            drt = {}

            def dtile(z, nm):
                if (z, nm) not in drt:
                    drt[(z, nm)] = dr.tile([128, BS], bf16, tag=nm,
                                           name=f"{nm}{z}")
                return drt[(z, nm)]

            outv = out_d[:, :].rearrange("(z p j) b -> z p j b", p=128, j=4)

            def drain_tail(z, ot, hwdge):
                """c(g3), o0, o2 after g1/g2-derived c_, with a_, b_ ready."""
                a_, b_, c_, d_ = (dtile(z, n) for n in "abcd")
                nc.vector.tensor_sub(out=c_, in0=dtile(z, "v"), in1=pst(z, 4))
                nc.vector.tensor_add(out=ot[:, 0], in0=a_, in1=c_)
                nc.vector.tensor_sub(out=ot[:, 2], in0=a_, in1=c_)
                if hwdge:
                    hwdge[0].dma_start(out=outv[z][:, 0], in_=ot[:, 0])
                    hwdge[2].dma_start(out=outv[z][:, 2], in_=ot[:, 2])
                else:
                    nc.gpsimd.dma_start(out=outv[z], in_=ot)

            # z0 + z1 partial, interleaved over xc
            ot0 = op_.tile([128, 4, BS], bf16, tag="ot", name="ot0")
            for c in range(XC):
                st, sp = c == 0, c == XC - 1
                for k in KORDER:
                    nc.tensor.matmul(pst(0, k), et[k][0][:, c], xt[k][c],
                                     start=st, stop=sp)
                if c >= 2:     # z1's E lands after X pair0; start its
                    for k in (3, 1):   # chains at chunk 2, wrap at the end
                        nc.tensor.matmul(pst(1, k), et[k][1][:, c], xt[k][c],
                                         start=c == 2, stop=False)
                filler(FILL0)
            for c in (0, 1):
                for k in (3, 1):
                    nc.tensor.matmul(pst(1, k), et[k][1][:, c], xt[k][c],
                                     start=False, stop=c == 1)
            filler(FILLB)
            # z0 drain (full, at end)
            t0, v0 = dtile(0, "t"), dtile(0, "v")
            a0, b0, c0, d0 = (dtile(0, n) for n in "abcd")
            nc.scalar.copy(out=t0, in_=pst(0, 0))
            nc.scalar.copy(out=v0, in_=pst(0, 1))
            nc.vector.tensor_sub(out=c0, in0=v0, in1=pst(0, 4))
            nc.vector.tensor_add(out=d0, in0=v0, in1=pst(0, 2))
            nc.vector.tensor_add(out=a0, in0=t0, in1=pst(0, 3))
            nc.vector.tensor_add(out=ot0[:, 0], in0=a0, in1=c0)
            nc.vector.tensor_sub(out=b0, in0=t0, in1=pst(0, 3))
            nc.vector.tensor_sub(out=ot0[:, 1], in0=b0, in1=d0)
            nc.vector.tensor_sub(out=ot0[:, 2], in0=a0, in1=c0)
            nc.vector.tensor_add(out=ot0[:, 3], in0=b0, in1=d0)
            nc.gpsimd.dma_start(out=outv[0], in_=ot0)
            derive(2)

            # z1: remaining chains (y0, g2, g3), drains interleaved
            ot1 = op_.tile([128, 4, BS], bf16, tag="ot", name="ot1")
            t1, v1 = dtile(1, "t"), dtile(1, "v")
            a1, b1, d1 = dtile(1, "a"), dtile(1, "b"), dtile(1, "d")
            nc.scalar.copy(out=v1, in_=pst(1, 1))
            for c in range(XC):
                nc.tensor.matmul(pst(1, 0), et[0][1][:, c], xt[0][c],
                                 start=c == 0, stop=c == XC - 1)
            nc.scalar.copy(out=t1, in_=pst(1, 0))
            nc.vector.tensor_add(out=a1, in0=t1, in1=pst(1, 3))
            nc.vector.tensor_sub(out=b1, in0=t1, in1=pst(1, 3))
            for c in range(XC):
                nc.tensor.matmul(pst(1, 2), et[2][1][:, c], xt[2][c],
                                 start=c == 0, stop=c == XC - 1)
            nc.vector.tensor_add(out=d1, in0=v1, in1=pst(1, 2))
            nc.vector.tensor_sub(out=ot1[:, 1], in0=b1, in1=d1)
            nc.vector.tensor_add(out=ot1[:, 3], in0=b1, in1=d1)
            for c in range(XC):
                nc.tensor.matmul(pst(1, 4), et[4][1][:, c], xt[4][c],
                                 start=c == 0, stop=c == XC - 1)
            drain_tail(1, ot1, None)
            derive(3)

            # z >= 2: k-major with interleaved drains
            for z in range(2, YZ):
                ot = op_.tile([128, 4, BS], bf16, tag="ot", name=f"ot{z}")
                oeng = {6: (nc.sync, nc.scalar, nc.sync, nc.scalar),
                        7: (nc.scalar, nc.sync, nc.scalar, nc.sync)}.get(z)
                t_, v_ = dtile(z, "t"), dtile(z, "v")
                a_, b_, d_ = dtile(z, "a"), dtile(z, "b"), dtile(z, "d")
                for k in KORDER_K:
                    for c in range(XC):
                        nc.tensor.matmul(pst(z, k), et[k][z][:, c], xt[k][c],
                                         start=c == 0, stop=c == XC - 1)
                    if k == 0:
                        nc.scalar.copy(out=t_, in_=pst(z, 0))    # Y0
                    elif k == 3:
                        nc.vector.tensor_add(out=a_, in0=t_, in1=pst(z, 3))
                        nc.vector.tensor_sub(out=b_, in0=t_, in1=pst(z, 3))
                    elif k == 1:
                        nc.scalar.copy(out=v_, in_=pst(z, 1))    # g1
                    elif k == 2:
                        nc.vector.tensor_add(out=d_, in0=v_, in1=pst(z, 2))
                        nc.vector.tensor_sub(out=ot[:, 1], in0=b_, in1=d_)
                        nc.vector.tensor_add(out=ot[:, 3], in0=b_, in1=d_)
                        if oeng:
                            oeng[1].dma_start(out=outv[z][:, 1], in_=ot[:, 1])
                            oeng[3].dma_start(out=outv[z][:, 3], in_=ot[:, 3])
                drain_tail(z, ot, oeng)
                if z + 2 < YZ:
                    derive(z + 2)
    nc.compile()
    return nc


def _prep_eigens(eigens):
    """eigens (gy, gx, 4) -> four (YZ, 128, XC, 128) bf16 E-matrices
    (E0, E1r, E1i, E2), [x, y] oriented, irfft scales folded."""
    e = np.ascontiguousarray(eigens.transpose(1, 0, 2)).astype(np.float32)  # (x,y,j)
    e0 = ((e[..., 0] + e[..., 2]) + (e[..., 1] + e[..., 3])) * 0.25
    e2 = ((e[..., 0] + e[..., 2]) - (e[..., 1] + e[..., 3])) * 0.25
    e1r = (e[..., 0] - e[..., 2]) * 0.5
    e1i = (e[..., 3] - e[..., 1]) * 0.5

    import ml_dtypes

    def chunk(m):  # (GX, GY) -> (YZ, 128, XC, 128): [z, p=x%128, c=x//128, q]
        return np.ascontiguousarray(
            m.reshape(XC, 128, YZ, 128).transpose(2, 1, 0, 3)
        ).astype(ml_dtypes.bfloat16)
    return (chunk(e0), chunk(e1r), chunk(e1i), chunk(e2))


def _prep_x(x):
    """x (B, IN) f32 -> four (XC, 128, B) bf16 bins (full batch; sliced
    per core in _in_maps)."""
    import ml_dtypes
    xb = np.ascontiguousarray(x, dtype=np.float32).reshape(B, GX, BLK)
    s02 = xb[:, :, 0] + xb[:, :, 2]
    s13 = xb[:, :, 1] + xb[:, :, 3]
    bins = (s02 + s13, xb[:, :, 0] - xb[:, :, 2],
            xb[:, :, 3] - xb[:, :, 1], s02 - s13)   # X0, X1r, X1i, X2
    return [np.ascontiguousarray(
                b.T.reshape(XC // 2, 2, 128, B).transpose(0, 2, 1, 3)
            ).astype(ml_dtypes.bfloat16) for b in bins]


def _in_maps(x, eigens):
    e0, e1r, e1i, e2 = _prep_eigens(np.asarray(eigens))
    x0, x1r, x1i, x2 = _prep_x(np.asarray(x))
    return [
        {"x0": np.ascontiguousarray(x0[:, :, :, c * BS:(c + 1) * BS]),
         "x1r": np.ascontiguousarray(x1r[:, :, :, c * BS:(c + 1) * BS]),
         "x1i": np.ascontiguousarray(x1i[:, :, :, c * BS:(c + 1) * BS]),
         "x2": np.ascontiguousarray(x2[:, :, :, c * BS:(c + 1) * BS]),
         "e0": e0, "e1r": e1r, "e1i": e1i, "e2": e2}
        for c in range(NCORES)
    ]


def _assemble(results):
    return np.concatenate(
        [np.asarray(r["outT"]).astype(np.float32).T for r in results], axis=0)


def kernel(x, eigens):
    from concourse.bass_utils import run_bass_kernel_spmd

    if "nc" not in _cache:
        _cache["nc"] = _build_nc()
    res = run_bass_kernel_spmd(_cache["nc"], _in_maps(x, eigens),
                               core_ids=list(range(NCORES)))
    return _assemble(res.results)


# revision 4
# speedup vs baseline: 1.0678x; 1.0051x over previous
"""Block-circulant linear (MINI_BLOCK=4) via length-4 rFFT factorization, v10.

Math: out = x @ W^T where W[4y+n, 4x+j] = eigens[y, x, (n-j) mod 4].
Length-4 DFT domain, Gauss 3-mult for the complex bin -> 5 real matmul
chains over gx=1024:
  X0 = x0+x1+x2+x3, X1 = (x0-x2) + i(x3-x1), X2 = x0-x1+x2-x3
  Y0 = X0 E0, Y2 = X2 E2, g1 = (X1r+X1i)E1r, g2 = X1r(E1i-E1r), g3 = X1i(E1r+E1i)
  Y1r = g1-g3, Y1i = g1+g2
  o0 = Y0+Y1r+Y2, o1 = Y0-Y1i-Y2, o2 = Y0-Y1r+Y2, o3 = Y0+Y1i-Y2

v5 notes (evidence-driven, from NTFF traces):
- Per-core aggregate DMA is ~250GB/s no matter how many queues, so total
  bytes rule the prologue. Ship only 4 E tensors (8MB) and derive
  Ed=E1i-E1r / Es=E1r+E1i on DVE per z-slice (+0.9us each, DVE has slack).
- TRN2 PE p-state: 0.65 -> 1.2 -> 2.4GHz, max only after ~3us of
  CONTINUOUS execution; any idle gap resets it. A warmup chain of dummy
  matmuls starts at the init barrier, and filler matmuls pad the
  DMA-starved xc-steps of z0/z1 so the clock never drops.
- Feature-major matmuls (stationary E [128x,128y], moving X [128x,512b]),
  contiguous bf16 drains (DVE strided writes are 4.5x slower), output
  transposed [feature, batch] fixed on host.
- k-major chain order for z>=2 with drains interleaved mid-group: only
  ~3us of combines left after the final matmul.
- Outs ride SWDGE (Pool) as 4x128KB per z; z6/z7 ride the HWDGE queues
  (idle by then).
"""
import numpy as np

B, IN, OUT, BLK = 4096, 4096, 4096, 4
GX, GY = IN // BLK, OUT // BLK        # 1024, 1024
NCORES = 8
BS = B // NCORES                      # 512 batch rows per core
XC = GX // 128                        # 8 contraction chunks
YZ = GY // 128                        # 8 output z-groups (128 y each)

_cache = {}

# bins: 0=Y0(E0*X0) 1=g1(E1r*X1s) 2=g2(Ed*X1r) 3=Y2(E2*X2) 4=g3(Es*X1i)
PS_BUFS = {0: 1, 1: 2, 2: 1, 3: 3, 4: 1}   # 8 banks; warmup shares the y2 ring
KORDER = (3, 2, 4, 0, 1)              # z0/z1 xc-major step order
KORDER_K = (0, 3, 1, 2, 4)            # k-major groups; drains interleave
WARMUP = 12                           # dummy matmuls before first real chain


def _build_nc():
    from concourse import bacc
    import concourse.mybir as mybir
    from concourse.tile import TileContext

    f32 = mybir.dt.float32
    bf16 = mybir.dt.bfloat16

    nc = bacc.Bacc("TRN2", target_bir_lowering=False, debug=False,
                   enable_asserts=False, num_devices=NCORES)
    x_d = [nc.dram_tensor(nm, [XC // 2, 128, 2, BS], bf16, kind="ExternalInput")
           for nm in ("x0", "x1r", "x1i", "x2")]
    e_d = [nc.dram_tensor(nm, [YZ, 128, XC, 128], bf16, kind="ExternalInput")
           for nm in ("e0", "e1r", "e1i", "e2")]
    out_d = nc.dram_tensor("outT", [OUT, BS], bf16, kind="ExternalOutput")

    with TileContext(nc) as tc:
        with (
            tc.tile_pool(name="xp", bufs=1) as xp,
            tc.tile_pool(name="ep", bufs=1) as ep,
            tc.tile_pool(name="dr", bufs=2) as dr,
            tc.tile_pool(name="op", bufs=4) as op_,
            tc.tile_pool(name="ps", bufs=1, space="PSUM") as ps,
        ):
            # ---- warmup: spin the PE p-state up while DMA streams in -----
            wt = xp.tile([128, BS], bf16, tag="wt")
            # warmup psum shares the y2 (ps3) ring: its buffer is reused by
            # z2's y2 chain, which only starts after the warmup chain ends.
            wpsum = ps.tile([128, BS], f32, tag="ps3", name="pswarm", bufs=3)
            nc.gpsimd.memset(wt, 0)
            for r in range(WARMUP):
                nc.tensor.matmul(wpsum, wt[:, 0:128], wt,
                                 start=r == 0, stop=r == WARMUP - 1)

            # ---- async loads (both HWDGE queues, need-order) --------------
            xpair = [[xp.tile([128, 2, BS], bf16, tag=f"xp{k}_{g}",
                               name=f"xp{k}_{g}") for g in range(XC // 2)]
                     for k in range(5)]  # k=1 slot is X1s (derived)
            xt = [[xpair[k][c // 2][:, c % 2] for c in range(XC)]
                  for k in range(5)]
            # et: k0=E0,k1=E1r,k2=Ed(derived),k3=E2,k4=Es(derived); el2=E1i
            et = [[ep.tile([128, XC, 128], bf16, tag=f"e{k}_{z}",
                           name=f"e{k}_{z}") for z in range(YZ)]
                  for k in range(5)]
            el2 = [ep.tile([128, XC, 128], bf16, tag=f"ei_{z}", name=f"ei_{z}")
                   for z in range(YZ)]

            def eload(k, z):       # k indexes (e0, e1r, e1i, e2)
                dst = el2[z] if k == 2 else et[k][z]
                return (dst, e_d[k][z])

            ez = [[eload(kk, z) for kk in ((3, 1, 2, 0) if z < 3
                                           else (0, 3, 1, 2))]
                  for z in range(YZ)]
            loads = [eload(3, 0), eload(1, 0), eload(2, 0), eload(0, 0)]
            loads += [(xpair[3][0], x_d[3][0]), (xpair[2][0], x_d[1][0]),
                      (xpair[4][0], x_d[2][0]), (xpair[0][0], x_d[0][0])]
            loads += ez[1][:2]                         # E2/E1r of z1 (interleaved bins)
            for g in range(1, XC // 2):                # X pairs + E z1/z2 JIT
                loads += [(xpair[3][g], x_d[3][g]), (xpair[2][g], x_d[1][g]),
                          (xpair[4][g], x_d[2][g]), (xpair[0][g], x_d[0][g])]
                if g == 1:
                    loads += ez[1][2:]
                elif g == 2:
                    loads += ez[2]
            # scalar (ACT) issues NO loads: HWDGE dma_start backpressure
            # on the ACT engine stream would delay the PSUM copies that
            # gate the single-buffered banks (seen as a 9us PE gap in v5).
            # Head (Ez0-2 + X) ping-pongs sync/SWDGE; late E slices ride
            # sync alone (it drains its head share by ~22us and lands z7
            # by ~55us, 20us before needed; SWDGE then turns to outs).
            hw = (nc.sync, nc.gpsimd)
            for i, (dst, src) in enumerate(loads):
                hw[i % 2].dma_start(out=dst, in_=src)
            for z in range(3, YZ):
                for dst, src in ez[z]:
                    nc.sync.dma_start(out=dst, in_=src)

            # early DVE work: Ed/Es for z0/z1, X1s per chunk
            def derive(z):
                nc.vector.tensor_sub(out=et[2][z], in0=el2[z], in1=et[1][z])
                nc.vector.tensor_add(out=et[4][z], in0=et[1][z], in1=el2[z])

            derive(0)
            for g in range(XC // 2):
                nc.vector.tensor_add(out=xpair[1][g], in0=xpair[2][g],
                                     in1=xpair[4][g])
            derive(1)

            # ---- main ------------------------------------------------------
            # Prologue: z0's 5 chains PLUS z1's two double-buffered bins
            # (y2, g1 -- loaded E only, no derives) interleave in one xc
            # sweep: 7 matmuls per chunk matches the ~3us/pair X arrival,
            # so the PE stays fed while X streams in. z1 finishes its
            # remaining 3 chains right after. z>=2: k-major, drains
            # interleaved mid-group.
            pstore = {}

            def pst(z, k):
                if (z, k) not in pstore:
                    pstore[(z, k)] = ps.tile([128, BS], f32, tag=f"ps{k}",
                                             name=f"ps{z}_{k}",
                                             bufs=PS_BUFS[k])
                return pstore[(z, k)]

            drt = {}

            def dtile(z, nm):
                if (z, nm) not in drt:
                    drt[(z, nm)] = dr.tile([128, BS], bf16, tag=nm,
                                           name=f"{nm}{z}")
                return drt[(z, nm)]

            outv = out_d[:, :].rearrange("(z p j) b -> z p j b", p=128, j=4)

            def drain_tail(z, ot, hwdge):
                """c(g3), o0, o2 after g1/g2-derived c_, with a_, b_ ready."""
                a_, b_, c_, d_ = (dtile(z, n) for n in "abcd")
                nc.vector.tensor_sub(out=c_, in0=dtile(z, "v"), in1=pst(z, 4))
                nc.vector.tensor_add(out=ot[:, 0], in0=a_, in1=c_)
                nc.vector.tensor_sub(out=ot[:, 2], in0=a_, in1=c_)
                if hwdge:
                    hwdge[0].dma_start(out=outv[z][:, 0], in_=ot[:, 0])
                    hwdge[2].dma_start(out=outv[z][:, 2], in_=ot[:, 2])
                else:
                    nc.gpsimd.dma_start(out=outv[z], in_=ot)

            # z0 + z1 partial, interleaved over xc
            ot0 = op_.tile([128, 4, BS], bf16, tag="ot", name="ot0")
            for c in range(XC):
                st, sp = c == 0, c == XC - 1
                for k in KORDER:
                    nc.tensor.matmul(pst(0, k), et[k][0][:, c], xt[k][c],
                                     start=st, stop=sp)
                if c >= 2:     # z1's E lands after X pair0; start its
                    for k in (3, 1):   # chains at chunk 2, wrap at the end
                        nc.tensor.matmul(pst(1, k), et[k][1][:, c], xt[k][c],
                                         start=c == 2, stop=False)
                if c >= 4:     # z2's y2 rides along once its E2 lands
                    nc.tensor.matmul(pst(2, 3), et[3][2][:, c], xt[3][c],
                                     start=c == 4, stop=False)
            for c in (0, 1):
                for k in (3, 1):
                    nc.tensor.matmul(pst(1, k), et[k][1][:, c], xt[k][c],
                                     start=False, stop=c == 1)
            for c in (0, 1, 2, 3):
                nc.tensor.matmul(pst(2, 3), et[3][2][:, c], xt[3][c],
                                 start=False, stop=c == 3)
            # z0 drain (full, at end)
            t0, v0 = dtile(0, "t"), dtile(0, "v")
            a0, b0, c0, d0 = (dtile(0, n) for n in "abcd")
            nc.scalar.copy(out=t0, in_=pst(0, 0))
            nc.scalar.copy(out=v0, in_=pst(0, 1))
            nc.vector.tensor_sub(out=c0, in0=v0, in1=pst(0, 4))
            nc.vector.tensor_add(out=d0, in0=v0, in1=pst(0, 2))
            nc.vector.tensor_add(out=a0, in0=t0, in1=pst(0, 3))
            nc.vector.tensor_add(out=ot0[:, 0], in0=a0, in1=c0)
            nc.vector.tensor_sub(out=b0, in0=t0, in1=pst(0, 3))
            nc.vector.tensor_sub(out=ot0[:, 1], in0=b0, in1=d0)
            nc.vector.tensor_sub(out=ot0[:, 2], in0=a0, in1=c0)
            nc.vector.tensor_add(out=ot0[:, 3], in0=b0, in1=d0)
            nc.gpsimd.dma_start(out=outv[0], in_=ot0)
            derive(2)

            # z1: remaining chains (y0, g2, g3), drains interleaved
            ot1 = op_.tile([128, 4, BS], bf16, tag="ot", name="ot1")
            t1, v1 = dtile(1, "t"), dtile(1, "v")
            a1, b1, d1 = dtile(1, "a"), dtile(1, "b"), dtile(1, "d")
            nc.scalar.copy(out=v1, in_=pst(1, 1))
            for c in range(XC):
                nc.tensor.matmul(pst(1, 0), et[0][1][:, c], xt[0][c],
                                 start=c == 0, stop=c == XC - 1)
            nc.scalar.copy(out=t1, in_=pst(1, 0))
            nc.vector.tensor_add(out=a1, in0=t1, in1=pst(1, 3))
            nc.vector.tensor_sub(out=b1, in0=t1, in1=pst(1, 3))
            for c in range(XC):
                nc.tensor.matmul(pst(1, 2), et[2][1][:, c], xt[2][c],
                                 start=c == 0, stop=c == XC - 1)
            nc.vector.tensor_add(out=d1, in0=v1, in1=pst(1, 2))
            nc.vector.tensor_sub(out=ot1[:, 1], in0=b1, in1=d1)
            nc.vector.tensor_add(out=ot1[:, 3], in0=b1, in1=d1)
            for c in range(XC):
                nc.tensor.matmul(pst(1, 4), et[4][1][:, c], xt[4][c],
                                 start=c == 0, stop=c == XC - 1)
            drain_tail(1, ot1, None)
            derive(3)

            # z >= 2: k-major with interleaved drains
            for z in range(2, YZ):
                ot = op_.tile([128, 4, BS], bf16, tag="ot", name=f"ot{z}")
                oeng = {6: (nc.sync, nc.scalar, nc.sync, nc.scalar),
                        7: (nc.scalar, nc.sync, nc.scalar, nc.sync)}.get(z)
                t_, v_ = dtile(z, "t"), dtile(z, "v")
                a_, b_, d_ = dtile(z, "a"), dtile(z, "b"), dtile(z, "d")
                for k in KORDER_K:
                    if z == 2 and k == 3:   # y2 ran during the prologue
                        nc.vector.tensor_add(out=a_, in0=t_, in1=pst(z, 3))
                        nc.vector.tensor_sub(out=b_, in0=t_, in1=pst(z, 3))
                        continue
                    for c in range(XC):
                        nc.tensor.matmul(pst(z, k), et[k][z][:, c], xt[k][c],
                                         start=c == 0, stop=c == XC - 1)
                    if k == 0:
                        nc.scalar.copy(out=t_, in_=pst(z, 0))    # Y0
                    elif k == 3:
                        nc.vector.tensor_add(out=a_, in0=t_, in1=pst(z, 3))
                        nc.vector.tensor_sub(out=b_, in0=t_, in1=pst(z, 3))
                    elif k == 1:
                        nc.scalar.copy(out=v_, in_=pst(z, 1))    # g1
                    elif k == 2:
                        nc.vector.tensor_add(out=d_, in0=v_, in1=pst(z, 2))
                        nc.vector.tensor_sub(out=ot[:, 1], in0=b_, in1=d_)
                        nc.vector.tensor_add(out=ot[:, 3], in0=b_, in1=d_)
                        if oeng:
                            oeng[1].dma_start(out=outv[z][:, 1], in_=ot[:, 1])
                            oeng[3].dma_start(out=outv[z][:, 3], in_=ot[:, 3])
                drain_tail(z, ot, oeng)
                if z + 2 < YZ:
                    derive(z + 2)
    nc.compile()
    return nc


def _prep_eigens(eigens):
    """eigens (gy, gx, 4) -> four (YZ, 128, XC, 128) bf16 E-matrices
    (E0, E1r, E1i, E2), [x, y] oriented, irfft scales folded."""
    e = np.ascontiguousarray(eigens.transpose(1, 0, 2)).astype(np.float32)  # (x,y,j)
    e0 = ((e[..., 0] + e[..., 2]) + (e[..., 1] + e[..., 3])) * 0.25
    e2 = ((e[..., 0] + e[..., 2]) - (e[..., 1] + e[..., 3])) * 0.25
    e1r = (e[..., 0] - e[..., 2]) * 0.5
    e1i = (e[..., 3] - e[..., 1]) * 0.5

    import ml_dtypes

    def chunk(m):  # (GX, GY) -> (YZ, 128, XC, 128): [z, p=x%128, c=x//128, q]
        return np.ascontiguousarray(
            m.reshape(XC, 128, YZ, 128).transpose(2, 1, 0, 3)
        ).astype(ml_dtypes.bfloat16)
    return (chunk(e0), chunk(e1r), chunk(e1i), chunk(e2))


def _prep_x(x):
    """x (B, IN) f32 -> four (XC, 128, B) bf16 bins (full batch; sliced
    per core in _in_maps)."""
    import ml_dtypes
    xb = np.ascontiguousarray(x, dtype=np.float32).reshape(B, GX, BLK)
    s02 = xb[:, :, 0] + xb[:, :, 2]
    s13 = xb[:, :, 1] + xb[:, :, 3]
    bins = (s02 + s13, xb[:, :, 0] - xb[:, :, 2],
            xb[:, :, 3] - xb[:, :, 1], s02 - s13)   # X0, X1r, X1i, X2
    return [np.ascontiguousarray(
                b.T.reshape(XC // 2, 2, 128, B).transpose(0, 2, 1, 3)
            ).astype(ml_dtypes.bfloat16) for b in bins]


def _in_maps(x, eigens):
    e0, e1r, e1i, e2 = _prep_eigens(np.asarray(eigens))
    x0, x1r, x1i, x2 = _prep_x(np.asarray(x))
    return [
        {"x0": np.ascontiguousarray(x0[:, :, :, c * BS:(c + 1) * BS]),
         "x1r": np.ascontiguousarray(x1r[:, :, :, c * BS:(c + 1) * BS]),
         "x1i": np.ascontiguousarray(x1i[:, :, :, c * BS:(c + 1) * BS]),
         "x2": np.ascontiguousarray(x2[:, :, :, c * BS:(c + 1) * BS]),
         "e0": e0, "e1r": e1r, "e1i": e1i, "e2": e2}
        for c in range(NCORES)
    ]


def _assemble(results):
    return np.concatenate(
        [np.asarray(r["outT"]).astype(np.float32).T for r in results], axis=0)


def kernel(x, eigens):
    from concourse.bass_utils import run_bass_kernel_spmd

    if "nc" not in _cache:
        _cache["nc"] = _build_nc()
    res = run_bass_kernel_spmd(_cache["nc"], _in_maps(x, eigens),
                               core_ids=list(range(NCORES)))
    return _assemble(res.results)
